# revision 9
# baseline (speedup 1.0000x reference)
"""Entmax-alpha (bisection) Trainium2 kernel.

Full inputs: att_scores [4,16,1024,1024] f32, alpha [16] f32.
Reference: p = entmax_bisect(att_scores, a) with a = 1.01 + 0.98*alpha per
head, 50 bisection iterations over the last axis (K=1024).

Key facts exploited here:
- In f32, the reference's bisection is bitwise-stationary after ~24
  iterations (dm < 0.5 ulp(tau)), so 26 evaluations reproduce the
  50-iteration result to ~1e-6 worst-row relative error.
- f_lo >= 0 always (the max element contributes exactly 1.0 at tau_lo), so
  the reference's `f_m * f_lo >= 0` condition reduces to `s_m >= 1`.
- dm_i = dm0 * 2^-i with dm0 = 1 - (1/K)^(1/(a-1)) is a per-(b,h)-slice
  constant, so the whole dm schedule is precomputed on host and shipped as
  a tiny per-slice constant table.
- pow(y, inv) = Exp(inv * Ln(y)); Ln and Exp live in the same ACT table
  set, and the Exp activation takes a per-partition scale (inv) and a free
  accumulator output (row sum) -> 2 ACT passes per bisection evaluation.

Sharding: 64 (b,h) slices, embarrassingly parallel -> 8 contiguous slices
per core across 8 NeuronCores.
"""

import numpy as np

import concourse.bacc as bacc
import concourse.tile as tile
from concourse import mybir
import concourse.bass as bass
from concourse.bass_utils import run_bass_kernel_spmd

F32 = np.float32

B, H, Q, K = 4, 16, 1024, 1024
N_CORES = 8
SLICES_PER_CORE = (B * H) // N_CORES  # 8
ROW_TILES = Q // 128  # 8
N_EVALS = 24
# scal columns: 0=am1(unused on-chip), 1=inv, 2=c0(=dm_1-1), 3..=dm_1..dm_N
M_COLS = 3 + N_EVALS

_AF = mybir.ActivationFunctionType
_OP = mybir.AluOpType

# --- ACT table-set selection fix -------------------------------------------
# Bacc's insert_act_table_loads picks, per activation, the FIRST table set
# containing that function: Ln -> "natural_log", Exp -> "exp_and_others".
# Alternating Ln/Exp then reloads tables every iteration (~1.3us each, ~47%
# of ScalarE time).  Both live in "natural_log_exp_and_others", so strip
# Ln/Exp from every other set's membership (indices into act_info.json are
# untouched) and the pass hoists a single load of the combined set.
_COMBINED_SET = "natural_log_exp_and_others"
_orig_gat = bacc.get_activation_tables


def _patched_gat(arch):
    tabs = _orig_gat(arch)
    out = {}
    for n, funcs in tabs.items():
        f = set(funcs)
        if n != _COMBINED_SET:
            f.discard(_AF.Ln)
            f.discard(_AF.Exp)
        out[n] = f
    return out


bacc.get_activation_tables = _patched_gat
# ---------------------------------------------------------------------------


def _build_nc(n_evals: int = N_EVALS):
    nc = bacc.Bacc("TRN2", target_bir_lowering=False, debug=False)
    x = nc.dram_tensor(
        "x", [SLICES_PER_CORE, Q, K], mybir.dt.float32, kind="ExternalInput"
    )
    scal = nc.dram_tensor(
        "scal", [SLICES_PER_CORE, M_COLS], mybir.dt.float32, kind="ExternalInput"
    )
    out = nc.dram_tensor(
        "out", [SLICES_PER_CORE, Q, K], mybir.dt.float32, kind="ExternalOutput"
    )

    GRP = 4  # independent row-tile chains interleaved per emission group
    # y and l are GROUP-wide [128, GRP*K] buffers: the 4 chains' relu-subs
    # write disjoint K-slices, then ONE wide Ln covers the whole group
    # ((224+4096)cyc vs 4*(224+1024)cyc -> ~14% fewer ACT cycles on Ln).

    with tile.TileContext(nc) as tc:
        with (
            tc.tile_pool(name="consts", bufs=1) as consts,
            tc.tile_pool(name="xr", bufs=GRP + 2) as xr_pool,
            tc.tile_pool(name="xs", bufs=GRP + 2) as xs_pool,
            tc.tile_pool(name="y", bufs=2) as y_pool,
            tc.tile_pool(name="l", bufs=2) as l_pool,
            tc.tile_pool(name="p", bufs=2 * GRP) as p_pool,
            tc.tile_pool(name="o", bufs=GRP + 1) as o_pool,
            tc.tile_pool(name="small", bufs=4 * GRP) as small,
        ):
            scal_sb = consts.tile([128, SLICES_PER_CORE, M_COLS], mybir.dt.float32)
            # broadcast the [S, M] table across all 128 partitions
            scal_bcast = bass.AP(
                tensor=scal[:].tensor,
                offset=0,
                ap=[[0, 128], [M_COLS, SLICES_PER_CORE], [1, M_COLS]],
            )
            nc.sync.dma_start(out=scal_sb, in_=scal_bcast)

            tiny = consts.tile([128, 1], mybir.dt.float32)
            nc.vector.memset(tiny, 1e-30)

            def dm_ap(s, i):  # dm_i of slice s, i in 1..n_evals
                return scal_sb[:, s, 2 + i : 3 + i]

            all_tiles = [
                (s, t) for s in range(SLICES_PER_CORE) for t in range(ROW_TILES)
            ]
            for g0 in range(0, len(all_tiles), GRP):
                group = all_tiles[g0 : g0 + GRP]
                st = {}  # (s,t) -> chain state

                for s, t in group:
                    rows = slice(t * 128, (t + 1) * 128)
                    x_t = xr_pool.tile([128, K], mybir.dt.float32, tag="xr")
                    nc.sync.dma_start(out=x_t, in_=x[s, rows, :])
                    xs_t = xs_pool.tile([128, K], mybir.dt.float32, tag="xs")
                    nc.vector.tensor_scalar_mul(
                        out=xs_t, in0=x_t, scalar1=scal_sb[:, s, 0:1]
                    )
                    mx = small.tile([128, 1], mybir.dt.float32, tag="mx")
                    nc.vector.reduce_max(out=mx, in_=xs_t, axis=mybir.AxisListType.X)
                    tau_lo = small.tile([128, 1], mybir.dt.float32, tag="tlo")
                    nc.vector.tensor_scalar_add(out=tau_lo, in0=mx, scalar1=-1.0)
                    tau_m = small.tile([128, 1], mybir.dt.float32, tag="tm")
                    nc.vector.tensor_scalar_add(
                        out=tau_m, in0=mx, scalar1=scal_sb[:, s, 2:3]
                    )
                    st[(s, t)] = {"xs": xs_t, "tlo": tau_lo, "tm": tau_m}

                ng = len(group)
                for i in range(1, n_evals + 1):
                    y_g = y_pool.tile([128, ng * K], mybir.dt.float32, tag="y")
                    for j, (s, t) in enumerate(group):
                        c = st[(s, t)]
                        nc.vector.tensor_scalar(
                            out=y_g[:, j * K : (j + 1) * K],
                            in0=c["xs"],
                            scalar1=c["tm"],
                            scalar2=0.0,
                            op0=_OP.subtract,
                            op1=_OP.max,
                        )
                    l_g = l_pool.tile([128, ng * K], mybir.dt.float32, tag="l")
                    nc.scalar.activation(
                        out=l_g, in_=y_g, func=_AF.Ln, bias=tiny, scale=1.0
                    )
                    for j, (s, t) in enumerate(group):
                        c = st[(s, t)]
                        p_t = p_pool.tile([128, K], mybir.dt.float32, tag="p")
                        s_t = small.tile([128, 1], mybir.dt.float32, tag="s")
                        nc.scalar.activation(
                            out=p_t,
                            in_=l_g[:, j * K : (j + 1) * K],
                            func=_AF.Exp,
                            bias=0.0,
                            scale=scal_sb[:, s, 1:2],
                            accum_out=s_t,
                        )
                        c["p"], c["s"] = p_t, s_t
                        if i < n_evals:
                            cond = small.tile([128, 1], mybir.dt.float32, tag="c")
                            nc.vector.tensor_scalar(
                                out=cond,
                                in0=s_t,
                                scalar1=1.0,
                                scalar2=None,
                                op0=_OP.is_ge,
                            )
                            tau_lo2 = small.tile([128, 1], mybir.dt.float32, tag="tlo")
                            nc.vector.scalar_tensor_tensor(
                                out=tau_lo2,
                                in0=cond,
                                scalar=dm_ap(s, i),
                                in1=c["tlo"],
                                op0=_OP.mult,
                                op1=_OP.add,
                            )
                            tau_m = small.tile([128, 1], mybir.dt.float32, tag="tm")
                            nc.vector.tensor_scalar_add(
                                out=tau_m, in0=tau_lo2, scalar1=dm_ap(s, i + 1)
                            )
                            c["tlo"], c["tm"] = tau_lo2, tau_m

                for s, t in group:
                    rows = slice(t * 128, (t + 1) * 128)
                    c = st[(s, t)]
                    r_t = small.tile([128, 1], mybir.dt.float32, tag="r")
                    nc.vector.reciprocal(out=r_t, in_=c["s"])
                    o_t = o_pool.tile([128, K], mybir.dt.float32, tag="o")
                    nc.vector.tensor_scalar_mul(out=o_t, in0=c["p"], scalar1=r_t)
                    nc.sync.dma_start(out=out[s, rows, :], in_=o_t)

    nc.finalize()
    return nc


_NC_CACHE = {}


def _get_nc():
    if "nc" not in _NC_CACHE:
        _NC_CACHE["nc"] = _build_nc()
    return _NC_CACHE["nc"]


def _host_scal_table(alpha: np.ndarray) -> np.ndarray:
    """Per-(b,h)-slice constant table, f32 math matching the reference."""
    a = (F32(1.01) + F32(0.98) * alpha.astype(F32)).astype(F32)  # [H]
    am1 = (a - F32(1.0)).astype(F32)
    inv = (F32(1.0) / am1).astype(F32)
    powd = (F32(1.0 / K) ** inv).astype(F32)  # (1/d)^inv
    dm0 = (F32(1.0) - powd).astype(F32)
    tab = np.zeros((B * H, M_COLS), dtype=F32)
    for g in range(B * H):
        h = g % H
        dms = [dm0[h]]
        for _ in range(N_EVALS):
            dms.append(F32(dms[-1] * F32(0.5)))
        tab[g, 0] = am1[h]
        tab[g, 1] = inv[h]
        tab[g, 2] = F32(dms[1] - F32(1.0))  # c0 = dm_1 - 1
        for i in range(1, N_EVALS + 1):
            tab[g, 2 + i] = dms[i]
    return tab


def kernel(att_scores: np.ndarray, alpha: np.ndarray, **run_kwargs) -> np.ndarray:
    assert att_scores.shape == (B, H, Q, K), att_scores.shape
    nc = _get_nc()
    xr = np.ascontiguousarray(att_scores, dtype=np.float32).reshape(B * H, Q, K)
    tab = _host_scal_table(np.asarray(alpha))
    in_maps = []
    for c in range(N_CORES):
        sl = slice(c * SLICES_PER_CORE, (c + 1) * SLICES_PER_CORE)
        in_maps.append(
            {
                "x": np.ascontiguousarray(xr[sl]),
                "scal": np.ascontiguousarray(tab[sl]),
            }
        )
    res = run_bass_kernel_spmd(nc, in_maps, core_ids=list(range(N_CORES)), **run_kwargs)
    outs = np.stack([res.results[c]["out"] for c in range(N_CORES)], axis=0)
    full = outs.reshape(B, H, Q, K).astype(np.float32)
    if run_kwargs:
        # expose profiling info to callers that asked for it (test harness)
        kernel.last_result = res
    return full


# revision 11
# speedup vs baseline: 1.1897x; 1.1897x over previous
"""Entmax-alpha (bisection) Trainium2 kernel.

Full inputs: att_scores [4,16,1024,1024] f32, alpha [16] f32.
Reference: p = entmax_bisect(att_scores, a) with a = 1.01 + 0.98*alpha per
head, 50 bisection iterations over the last axis (K=1024).

Key facts exploited here:
- In f32, the reference's bisection is bitwise-stationary after ~24
  iterations (dm < 0.5 ulp(tau)), so 26 evaluations reproduce the
  50-iteration result to ~1e-6 worst-row relative error.
- f_lo >= 0 always (the max element contributes exactly 1.0 at tau_lo), so
  the reference's `f_m * f_lo >= 0` condition reduces to `s_m >= 1`.
- dm_i = dm0 * 2^-i with dm0 = 1 - (1/K)^(1/(a-1)) is a per-(b,h)-slice
  constant, so the whole dm schedule is precomputed on host and shipped as
  a tiny per-slice constant table.
- pow(y, inv) = Exp(inv * Ln(y)); Ln and Exp live in the same ACT table
  set, and the Exp activation takes a per-partition scale (inv) and a free
  accumulator output (row sum) -> 2 ACT passes per bisection evaluation.

Sharding: 64 (b,h) slices, embarrassingly parallel -> 8 contiguous slices
per core across 8 NeuronCores.
"""

import numpy as np

import concourse.bacc as bacc
import concourse.tile as tile
from concourse import mybir
import concourse.bass as bass
from concourse.bass_utils import run_bass_kernel_spmd

F32 = np.float32

B, H, Q, K = 4, 16, 1024, 1024
N_CORES = 8
SLICES_PER_CORE = (B * H) // N_CORES  # 8
ROW_TILES = Q // 128  # 8
N_EVALS = 24
# scal columns: 0=am1(unused on-chip), 1=inv, 2=c0(=dm_1-1), 3..=dm_1..dm_N
M_COLS = 3 + N_EVALS

_AF = mybir.ActivationFunctionType
_OP = mybir.AluOpType

# --- ACT table-set selection fix -------------------------------------------
# Bacc's insert_act_table_loads picks, per activation, the FIRST table set
# containing that function: Ln -> "natural_log", Exp -> "exp_and_others".
# Alternating Ln/Exp then reloads tables every iteration (~1.3us each, ~47%
# of ScalarE time).  Both live in "natural_log_exp_and_others", so strip
# Ln/Exp from every other set's membership (indices into act_info.json are
# untouched) and the pass hoists a single load of the combined set.
_COMBINED_SET = "natural_log_exp_and_others"
_orig_gat = bacc.get_activation_tables


def _patched_gat(arch):
    tabs = _orig_gat(arch)
    out = {}
    for n, funcs in tabs.items():
        f = set(funcs)
        if n != _COMBINED_SET:
            f.discard(_AF.Ln)
            f.discard(_AF.Exp)
        out[n] = f
    return out


bacc.get_activation_tables = _patched_gat
# ---------------------------------------------------------------------------


def _build_nc(n_evals: int = N_EVALS):
    nc = bacc.Bacc("TRN2", target_bir_lowering=False, debug=False)
    x = nc.dram_tensor(
        "x", [SLICES_PER_CORE, Q, K], mybir.dt.float32, kind="ExternalInput"
    )
    scal = nc.dram_tensor(
        "scal", [SLICES_PER_CORE, M_COLS], mybir.dt.float32, kind="ExternalInput"
    )
    out = nc.dram_tensor(
        "out", [SLICES_PER_CORE, Q, K], mybir.dt.float32, kind="ExternalOutput"
    )

    GRP = 4  # independent row-tile chains interleaved per emission group
    # y and l are GROUP-wide [128, GRP*K] buffers: the 4 chains' relu-subs
    # write disjoint K-slices, then ONE wide Ln covers the whole group
    # ((224+4096)cyc vs 4*(224+1024)cyc -> ~14% fewer ACT cycles on Ln).

    with tile.TileContext(nc) as tc:
        with (
            tc.tile_pool(name="consts", bufs=1) as consts,
            tc.tile_pool(name="xr", bufs=GRP + 2) as xr_pool,
            tc.tile_pool(name="xs", bufs=GRP + 2) as xs_pool,
            tc.tile_pool(name="y", bufs=4) as y_pool,
            tc.tile_pool(name="l", bufs=4) as l_pool,
            tc.tile_pool(name="p", bufs=2 * GRP) as p_pool,
            tc.tile_pool(name="o", bufs=GRP + 1) as o_pool,
            tc.tile_pool(name="small", bufs=4 * GRP) as small,
        ):
            scal_sb = consts.tile([128, SLICES_PER_CORE, M_COLS], mybir.dt.float32)
            # broadcast the [S, M] table across all 128 partitions
            scal_bcast = bass.AP(
                tensor=scal[:].tensor,
                offset=0,
                ap=[[0, 128], [M_COLS, SLICES_PER_CORE], [1, M_COLS]],
            )
            nc.sync.dma_start(out=scal_sb, in_=scal_bcast)

            tiny = consts.tile([128, 1], mybir.dt.float32)
            nc.vector.memset(tiny, 1e-30)

            def dm_ap(s, i):  # dm_i of slice s, i in 1..n_evals
                return scal_sb[:, s, 2 + i : 3 + i]

            all_tiles = [
                (s, t) for s in range(SLICES_PER_CORE) for t in range(ROW_TILES)
            ]
            for g0 in range(0, len(all_tiles), GRP):
                group = all_tiles[g0 : g0 + GRP]
                st = {}  # (s,t) -> chain state

                for s, t in group:
                    rows = slice(t * 128, (t + 1) * 128)
                    x_t = xr_pool.tile([128, K], mybir.dt.float32, tag="xr")
                    nc.sync.dma_start(out=x_t, in_=x[s, rows, :])
                    xs_t = xs_pool.tile([128, K], mybir.dt.float32, tag="xs")
                    nc.vector.tensor_scalar_mul(
                        out=xs_t, in0=x_t, scalar1=scal_sb[:, s, 0:1]
                    )
                    mx = small.tile([128, 1], mybir.dt.float32, tag="mx")
                    nc.vector.reduce_max(out=mx, in_=xs_t, axis=mybir.AxisListType.X)
                    tau_lo = small.tile([128, 1], mybir.dt.float32, tag="tlo")
                    nc.vector.tensor_scalar_add(out=tau_lo, in0=mx, scalar1=-1.0)
                    tau_m = small.tile([128, 1], mybir.dt.float32, tag="tm")
                    nc.vector.tensor_scalar_add(
                        out=tau_m, in0=mx, scalar1=scal_sb[:, s, 2:3]
                    )
                    st[(s, t)] = {"xs": xs_t, "tlo": tau_lo, "tm": tau_m}

                ng = len(group)
                for i in range(1, n_evals + 1):
                    for q0 in range(0, ng, 2):
                        pair = group[q0 : q0 + 2]
                        np_ = len(pair)
                        y_g = y_pool.tile([128, np_ * K], mybir.dt.float32, tag="y")
                        for j, (s, t) in enumerate(pair):
                            c = st[(s, t)]
                            nc.vector.tensor_scalar(
                                out=y_g[:, j * K : (j + 1) * K],
                                in0=c["xs"],
                                scalar1=c["tm"],
                                scalar2=0.0,
                                op0=_OP.subtract,
                                op1=_OP.max,
                            )
                        l_g = l_pool.tile([128, np_ * K], mybir.dt.float32, tag="l")
                        nc.scalar.activation(
                            out=l_g, in_=y_g, func=_AF.Ln, bias=tiny, scale=1.0
                        )
                        for j, (s, t) in enumerate(pair):
                            c = st[(s, t)]
                            p_t = p_pool.tile([128, K], mybir.dt.float32, tag="p")
                            s_t = small.tile([128, 1], mybir.dt.float32, tag="s")
                            nc.scalar.activation(
                                out=p_t,
                                in_=l_g[:, j * K : (j + 1) * K],
                                func=_AF.Exp,
                                bias=0.0,
                                scale=scal_sb[:, s, 1:2],
                                accum_out=s_t,
                            )
                            c["p"], c["s"] = p_t, s_t
                            if i < n_evals:
                                cond = small.tile([128, 1], mybir.dt.float32, tag="c")
                                nc.vector.tensor_scalar(
                                    out=cond,
                                    in0=s_t,
                                    scalar1=1.0,
                                    scalar2=None,
                                    op0=_OP.is_ge,
                                )
                                tau_lo2 = small.tile(
                                    [128, 1], mybir.dt.float32, tag="tlo"
                                )
                                nc.vector.scalar_tensor_tensor(
                                    out=tau_lo2,
                                    in0=cond,
                                    scalar=dm_ap(s, i),
                                    in1=c["tlo"],
                                    op0=_OP.mult,
                                    op1=_OP.add,
                                )
                                tau_m = small.tile([128, 1], mybir.dt.float32, tag="tm")
                                nc.vector.tensor_scalar_add(
                                    out=tau_m, in0=tau_lo2, scalar1=dm_ap(s, i + 1)
                                )
                                c["tlo"], c["tm"] = tau_lo2, tau_m

                for s, t in group:
                    rows = slice(t * 128, (t + 1) * 128)
                    c = st[(s, t)]
                    r_t = small.tile([128, 1], mybir.dt.float32, tag="r")
                    nc.vector.reciprocal(out=r_t, in_=c["s"])
                    o_t = o_pool.tile([128, K], mybir.dt.float32, tag="o")
                    nc.vector.tensor_scalar_mul(out=o_t, in0=c["p"], scalar1=r_t)
                    nc.sync.dma_start(out=out[s, rows, :], in_=o_t)

    nc.finalize()
    return nc


_NC_CACHE = {}


def _get_nc():
    if "nc" not in _NC_CACHE:
        _NC_CACHE["nc"] = _build_nc()
    return _NC_CACHE["nc"]


def _host_scal_table(alpha: np.ndarray) -> np.ndarray:
    """Per-(b,h)-slice constant table, f32 math matching the reference."""
    a = (F32(1.01) + F32(0.98) * alpha.astype(F32)).astype(F32)  # [H]
    am1 = (a - F32(1.0)).astype(F32)
    inv = (F32(1.0) / am1).astype(F32)
    powd = (F32(1.0 / K) ** inv).astype(F32)  # (1/d)^inv
    dm0 = (F32(1.0) - powd).astype(F32)
    tab = np.zeros((B * H, M_COLS), dtype=F32)
    for g in range(B * H):
        h = g % H
        dms = [dm0[h]]
        for _ in range(N_EVALS):
            dms.append(F32(dms[-1] * F32(0.5)))
        tab[g, 0] = am1[h]
        tab[g, 1] = inv[h]
        tab[g, 2] = F32(dms[1] - F32(1.0))  # c0 = dm_1 - 1
        for i in range(1, N_EVALS + 1):
            tab[g, 2 + i] = dms[i]
    return tab


def kernel(att_scores: np.ndarray, alpha: np.ndarray, **run_kwargs) -> np.ndarray:
    assert att_scores.shape == (B, H, Q, K), att_scores.shape
    nc = _get_nc()
    xr = np.ascontiguousarray(att_scores, dtype=np.float32).reshape(B * H, Q, K)
    tab = _host_scal_table(np.asarray(alpha))
    in_maps = []
    for c in range(N_CORES):
        sl = slice(c * SLICES_PER_CORE, (c + 1) * SLICES_PER_CORE)
        in_maps.append(
            {
                "x": np.ascontiguousarray(xr[sl]),
                "scal": np.ascontiguousarray(tab[sl]),
            }
        )
    res = run_bass_kernel_spmd(nc, in_maps, core_ids=list(range(N_CORES)), **run_kwargs)
    outs = np.stack([res.results[c]["out"] for c in range(N_CORES)], axis=0)
    full = outs.reshape(B, H, Q, K).astype(np.float32)
    if run_kwargs:
        # expose profiling info to callers that asked for it (test harness)
        kernel.last_result = res
    return full
